# revision 17
# baseline (speedup 1.0000x reference)
"""GQA attention (B=2, T=2048, DIM=2048, NH=32, NKV=8, HD=64) with RoPE, causal,
on 8 TRN2 NeuronCores.

Sharding: data-parallel over B (2) x tensor-parallel over kv-head groups (4).
Core i handles batch i//4 and kv heads {2g, 2g+1} (g = i%4), i.e. q heads
8g..8g+8. wq/wk/wv column-parallel, wo row-parallel; host sums the 4 partial
outputs per batch.

Everything on-device is feature-major ("transposed"): x^T, Q^T, K^T are
[feature, t] so no on-device transposes are needed anywhere:
  QT[d,t] = wq^T x^T;  scoresT[s,q] = (KT slice)^T @ QT;  OT[d,q] = V^T @ PT;
  outT[o,t] = wo^T @ OT.  Host transposes the final [o,t] back to [t,o].

RoPE de-interleave: within each head the 64 features are permuted to
[32 real | 32 imag] (host permutes wq/wk columns), so rot = q*C + swap(q)*S
with the half-swap done by SBUF->SBUF DMA.

Head->row layout: slab s (of 4) holds q heads (8g+s) at rows 0:64 and
(8g+s+4) at rows 64:128, so the two heads of a slab use kv0/kv1 = rows
0:64/64:128 of KT directly, and the two score matmuls of a slab pack into
PE row-groups (0,0)/(64,0) and run concurrently.

Softmax: no max subtraction (|score| <~ 6 after the 1/8 scale folded into wq).
Denominator via an appended ones-column on V (row 64 of the PV accumulator);
1/denom via reciprocal_approx_fast + gpsimd partition_broadcast. Causal mask:
DVE multiply by a host-precomputed 0/1 tile on diagonal score pairs only;
fully-masked tiles are skipped. Scores are computed in side-by-side pairs in a
2-bank PSUM tensor so one ACT exp instruction covers 1024 columns.

The per-t-block projection, attention, and output-projection stages are
emitted in one flat loop so the Tile scheduler overlaps PE-bound projection /
wo matmuls with the ACT-bound exp stream of the attention stage.

All matmul operands are fp16 (KERNEL_MM_DTYPE also allows bf16/f32r); the PE
streams 2-byte operands at full 2.4 GHz rate, fp32 PSUM accumulate.
"""

import numpy as np

B, T, DIM = 2, 2048, 2048
NH, NKV, HD = 32, 8, 64
G = 4            # tensor-parallel groups
QH = NH // G     # 8 local q heads
SLABS = 4
KTILES = DIM // 128
TBLK = T // 512

_CACHE = {}
import os as _os
_MM_DTYPE = _os.environ.get("KERNEL_MM_DTYPE", "fp16")


def _to_mm_dtype(x: np.ndarray) -> np.ndarray:
    if _MM_DTYPE == "bf16":
        import ml_dtypes
        return np.ascontiguousarray(x, dtype=np.float32).astype(ml_dtypes.bfloat16)
    if _MM_DTYPE == "fp16":
        return np.ascontiguousarray(x, dtype=np.float32).astype(np.float16)
    return _round_f32r(x)


def _round_f32r(x: np.ndarray) -> np.ndarray:
    """Round f32 to the float32r grid (11 mantissa bits, round-to-nearest-even)."""
    x = np.ascontiguousarray(x, dtype=np.float32)
    xi = x.view(np.uint32).copy()
    shift = 12  # keep 11 mantissa bits
    lsb = (xi >> shift) & 1
    xi = (xi + ((1 << (shift - 1)) - 1) + lsb) & np.uint32(~((1 << shift) - 1) & 0xFFFFFFFF)
    return xi.view(np.float32)


def _build():
    import concourse.bass as bass
    import concourse.mybir as mybir
    import concourse.tile as tile
    from concourse import bacc

    F32 = mybir.dt.float32
    F32R = {"bf16": mybir.dt.bfloat16, "fp16": mybir.dt.float16,
            "f32r": mybir.dt.float32r}[_MM_DTYPE]
    EXP = mybir.ActivationFunctionType.Exp

    nc = bacc.Bacc("TRN2", target_bir_lowering=False, debug=False, num_devices=8)

    xT = nc.dram_tensor("xT", [DIM, T], F32R, kind="ExternalInput").ap()
    wq = nc.dram_tensor("wq", [DIM, QH * HD], F32R, kind="ExternalInput").ap()
    wk = nc.dram_tensor("wk", [DIM, 2 * HD], F32R, kind="ExternalInput").ap()
    wv = nc.dram_tensor("wv", [DIM, 2 * HD], F32R, kind="ExternalInput").ap()
    wo = nc.dram_tensor("wo", [QH * HD, DIM], F32R, kind="ExternalInput").ap()
    c4 = nc.dram_tensor("c4", [128, T], F32, kind="ExternalInput").ap()
    s4 = nc.dram_tensor("s4", [128, T], F32, kind="ExternalInput").ap()
    vones = nc.dram_tensor("vones", [128, 16 * 65], F32R, kind="ExternalInput").ap()
    msk = nc.dram_tensor("msk", [128, 2048], F32R, kind="ExternalInput").ap()
    outT = nc.dram_tensor("outT", [DIM, T], F32, kind="ExternalOutput").ap()

    from contextlib import ExitStack

    with tile.TileContext(nc) as tc, ExitStack() as ctx:
        # ---------- persistent tiles ----------
        pers = ctx.enter_context(tc.tile_pool(name="pers", bufs=1))
        KT = pers.tile([128, T], F32R, tag="kt", name="kt")
        V0 = pers.tile([128, 16 * 65], F32R, tag="v0", name="v0")
        V1 = pers.tile([128, 16 * 65], F32R, tag="v1", name="v1")
        MSK = pers.tile([128, 2048], F32R, tag="msk", name="msk_sb")
        C4 = pers.tile([128, T], F32, tag="c4", name="c4_sb")
        S4 = pers.tile([128, T], F32, tag="s4", name="s4_sb")
        WQ = pers.tile([128, KTILES * 512], F32R, tag="wq", name="wq_sb")
        WK = pers.tile([128, KTILES * 128], F32R, tag="wk", name="wk_sb")
        WV = pers.tile([128, KTILES * 128], F32R, tag="wv", name="wv_sb")
        WO = [pers.tile([128, T], F32R, tag=f"wo{s}", name=f"wo{s}") for s in range(SLABS)]

        rot = ctx.enter_context(tc.tile_pool(name="rot", bufs=3))
        work = ctx.enter_context(tc.tile_pool(name="work", bufs=2))
        ptp = ctx.enter_context(tc.tile_pool(name="ptp", bufs=7))
        misc = ctx.enter_context(tc.tile_pool(name="misc", bufs=3))
        osbp = ctx.enter_context(tc.tile_pool(name="osbp", bufs=6))
        xtp = ctx.enter_context(tc.tile_pool(name="xt", bufs=2))
        ps_acc = ctx.enter_context(tc.tile_pool(name="ps_acc", bufs=2, space="PSUM"))
        ps_po = ps_acc
        ps_sc = ctx.enter_context(tc.tile_pool(name="ps_sc", bufs=2, space="PSUM"))
        ps_ot = ctx.enter_context(tc.tile_pool(name="ps_ot", bufs=2, space="PSUM"))

        # first x block, then q weights, then the rest in need-order
        xts0 = []
        for k in range(KTILES):
            xt_t = xtp.tile([128, 512], F32R, tag=f"x{k}", name=f"xt0_{k}")
            nc.sync.dma_start(xt_t[:], xT[k * 128:(k + 1) * 128, 0:512])
            xts0.append(xt_t)
        for k in range(KTILES):
            nc.scalar.dma_start(WQ[:, k * 512:(k + 1) * 512], wq[k * 128:(k + 1) * 128, :])
        for k in range(KTILES):
            nc.gpsimd.dma_start(WK[:, k * 128:(k + 1) * 128], wk[k * 128:(k + 1) * 128, :])
            nc.gpsimd.dma_start(WV[:, k * 128:(k + 1) * 128], wv[k * 128:(k + 1) * 128, :])
        nc.gpsimd.dma_start(C4[:], c4[:])
        nc.gpsimd.dma_start(S4[:], s4[:])
        nc.gpsimd.dma_start(V0[:], vones[:])
        nc.gpsimd.dma_start(V1[:], vones[:])
        nc.gpsimd.dma_start(MSK[:], msk[:])
        for s in range(SLABS):
            nc.gpsimd.dma_start(WO[s][:], wo[s * 128:(s + 1) * 128, :])

        for tb in range(TBLK):
            t_sl = slice(tb * 512, (tb + 1) * 512)
            # ---- x tiles for this t block (tb=0 preloaded above) ----
            if tb == 0:
                xts = xts0
            else:
                xts = []
                for k in range(KTILES):
                    xt_t = xtp.tile([128, 512], F32R, tag=f"x{k}", name=f"xt{tb}_{k}")
                    nc.sync.dma_start(xt_t[:], xT[k * 128:(k + 1) * 128, t_sl])
                    xts.append(xt_t)

            # ---- projections + rope for this t block ----
            QTr = []
            for s in range(SLABS + 1):
                ps = ps_acc.tile([128, 512], F32, tag="acc", name="pq")
                for k in range(KTILES):
                    if s < SLABS:
                        lhs = WQ[:, k * 512 + s * 128: k * 512 + (s + 1) * 128]
                    else:
                        lhs = WK[:, k * 128:(k + 1) * 128]
                    nc.tensor.matmul(ps[:], lhs, xts[k][:],
                                     start=(k == 0), stop=(k == KTILES - 1))
                if s < SLABS:
                    dst_t = rot.tile([128, 512], F32R, tag=f"qtr{s}", name=f"qtr{s}")
                    QTr.append(dst_t)
                    dst = dst_t[:]
                else:
                    dst = KT[:, t_sl]
                q_sb = work.tile([128, 512], F32, tag="qsb", name="qsb")
                nc.vector.tensor_copy(q_sb[:], ps[:])
                q_sw = work.tile([128, 512], F32, tag="qsw", name="qsw")
                for o in (0, 64):
                    nc.gpsimd.dma_start(q_sw[o:o + 32, :], q_sb[o + 32:o + 64, :])
                    nc.gpsimd.dma_start(q_sw[o + 32:o + 64, :], q_sb[o:o + 32, :])
                m1 = work.tile([128, 512], F32, tag="m1", name="m1")
                nc.vector.tensor_mul(m1[:], ps[:], C4[:, t_sl])
                m2 = work.tile([128, 512], F32, tag="m2", name="m2")
                nc.vector.tensor_mul(m2[:], q_sw[:], S4[:, t_sl])
                nc.vector.tensor_add(dst, m1[:], m2[:])
            # ---- V for this t block ----
            for i in range(4):
                sbi = tb * 4 + i
                pv = ps_acc.tile([128, 128], F32, tag="acc", name="pv",
                                 padded_shape=[128, 512])
                for k in range(KTILES):
                    nc.tensor.matmul(pv[:], xts[k][:, i * 128:(i + 1) * 128],
                                     WV[:, k * 128:(k + 1) * 128],
                                     start=(k == 0), stop=(k == KTILES - 1))
                nc.vector.tensor_copy(V0[:, sbi * 65: sbi * 65 + 64], pv[:, 0:64])
                nc.vector.tensor_copy(V1[:, sbi * 65: sbi * 65 + 64], pv[:, 64:128])

            # ---- attention for q chunk qc = tb ----
            qc = tb
            OTNr = [rot.tile([128, 512], F32R, tag=f"otnr{s}", name=f"otnr{s}")
                    for s in range(SLABS)]
            npair = (qc * 4 + 4) // 2
            for s in range(SLABS):
                for half in range(2):
                    hs = slice(half * 64, (half + 1) * 64)
                    V_ = V0 if half == 0 else V1
                    ot = ps_ot.tile([65, 512], mybir.dt.float32, tag="ot", name="ot")
                    for pr in range(npair):
                        sb0 = 2 * pr
                        sc = ps_sc.tile([128, 1024], mybir.dt.float32, tag="sc", name="sc")
                        nc.tensor.matmul(sc[:, 0:512],
                                         KT[hs, sb0 * 128:(sb0 + 1) * 128],
                                         QTr[s][hs, :], start=True, stop=True)
                        nc.tensor.matmul(sc[:, 512:1024],
                                         KT[hs, (sb0 + 1) * 128:(sb0 + 2) * 128],
                                         QTr[s][hs, :], start=True, stop=True)
                        pt = ptp.tile([128, 1024], F32R, tag="pt", name="pt")
                        nc.scalar.activation(pt[:], sc[:], EXP)
                        if sb0 + 1 >= qc * 4:  # pair touches the diagonal
                            v = (sb0 - qc * 4) // 2
                            nc.vector.tensor_mul(pt[:], pt[:],
                                                 MSK[:, v * 1024:(v + 1) * 1024])
                        nc.tensor.matmul(ot[:], V_[:, sb0 * 65: sb0 * 65 + 65],
                                         pt[:, 0:512],
                                         start=(pr == 0), stop=False)
                        nc.tensor.matmul(ot[:], V_[:, (sb0 + 1) * 65: (sb0 + 1) * 65 + 65],
                                         pt[:, 512:1024],
                                         start=False, stop=(pr == npair - 1))
                    dsb = misc.tile([1, 512], F32, tag="dsb", name="dsb")
                    nc.vector.tensor_copy(dsb[:], ot[64:65, :])
                    rcf = misc.tile([1, 512], F32, tag="rcf", name="rcf")
                    nc.vector.reciprocal_approx_fast(rcf[:], dsb[:])
                    bc = misc.tile([64, 512], F32, tag="bc", name="bc")
                    nc.gpsimd.partition_broadcast(bc[:], rcf[:])
                    nc.vector.tensor_mul(OTNr[s][hs, :], ot[0:64, :], bc[:])

            # ---- output projection for this t chunk ----
            for ob in range(16):
                po = ps_po.tile([128, 512], mybir.dt.float32, tag="acc", name="po")
                for s in range(SLABS):
                    nc.tensor.matmul(po[:], WO[s][:, ob * 128:(ob + 1) * 128],
                                     OTNr[s][:], start=(s == 0), stop=(s == SLABS - 1))
                osb = osbp.tile([128, 512], mybir.dt.float32, tag="osb", name="osb")
                nc.vector.tensor_copy(osb[:], po[:])
                nc.sync.dma_start(outT[ob * 128:(ob + 1) * 128, t_sl], osb[:])

    nc.compile()
    return nc


def _prep_inputs(x, freqs_cos, freqs_sin, wq, wk, wv, wo):
    """Build the 8 per-core input maps (host-side sharding + layout prep)."""
    x = np.asarray(x, dtype=np.float32)
    freqs_cos = np.asarray(freqs_cos, dtype=np.float32)
    freqs_sin = np.asarray(freqs_sin, dtype=np.float32)
    wq = np.asarray(wq, dtype=np.float32)
    wk = np.asarray(wk, dtype=np.float32)
    wv = np.asarray(wv, dtype=np.float32)
    wo = np.asarray(wo, dtype=np.float32)

    # de-interleave permutation within a head: [2j] then [2j+1]
    deint = np.concatenate([np.arange(0, HD, 2), np.arange(1, HD, 2)])

    # rope tables [128, T]: row r uses freq index r % 32; sign of sin flips
    # per 32-block (real-out blocks get -sin)
    cosT = freqs_cos.T  # [32, T]
    sinT = freqs_sin.T
    c4 = np.tile(cosT, (4, 1)).astype(np.float32)
    s4 = np.concatenate([-sinT, sinT, -sinT, sinT], axis=0).astype(np.float32)

    vones = np.zeros((128, 16 * 65), dtype=np.float32)
    vones[:, 64::65] = 1.0
    # pair masks: variant v covers s-offsets {256v, 256v+128} vs q in [0,512):
    # msk[p, v*1024 + j*512 + q] = 1 if (256v + 128j + p) <= q else 0
    msk = np.zeros((128, 2048), dtype=np.float32)
    p_ = np.arange(128)[:, None]
    q_ = np.arange(512)[None, :]
    for v in range(2):
        for j in range(2):
            blk = (256 * v + 128 * j + p_) <= q_
            msk[:, v * 1024 + j * 512:(v * 1024 + (j + 1) * 512)] = blk

    in_maps = []
    for core in range(8):
        b, g = divmod(core, 4)
        # local q head order: slab-major, (s, half) -> global head 8g + s + 4*half
        qheads = [8 * g + s + 4 * h for s in range(SLABS) for h in range(2)]
        kvheads = [2 * g, 2 * g + 1]

        wq_cols = np.concatenate([qh * HD + deint for qh in qheads])
        wk_cols = np.concatenate([kh * HD + deint for kh in kvheads])
        wv_cols = np.concatenate([np.arange(kh * HD, (kh + 1) * HD) for kh in kvheads])
        wo_rows = np.concatenate([np.arange(qh * HD, (qh + 1) * HD) for qh in qheads])

        in_maps.append({
            "xT": _to_mm_dtype(x[b].T),
            "wq": _to_mm_dtype(wq[:, wq_cols] * (1.0 / np.sqrt(HD))),
            "wk": _to_mm_dtype(wk[:, wk_cols]),
            "wv": _to_mm_dtype(wv[:, wv_cols]),
            "wo": _to_mm_dtype(wo[wo_rows, :]),
            "c4": c4,
            "s4": s4,
            "vones": _to_mm_dtype(vones),
            "msk": _to_mm_dtype(msk),
        })
    return in_maps


def kernel(x, freqs_cos, freqs_sin, wq, wk, wv, wo, _trace=False):
    from concourse.bass_utils import run_bass_kernel_spmd

    if "nc" not in _CACHE:
        _CACHE["nc"] = _build()
    nc = _CACHE["nc"]

    in_maps = _prep_inputs(x, freqs_cos, freqs_sin, wq, wk, wv, wo)
    res = run_bass_kernel_spmd(nc, in_maps, core_ids=list(range(8)), trace=_trace)
    _CACHE["last_result"] = res

    out = np.empty((B, T, DIM), dtype=np.float32)
    for b in range(B):
        acc = res.results[4 * b]["outT"].astype(np.float32).copy()
        for g in range(1, 4):
            acc += res.results[4 * b + g]["outT"]
        out[b] = acc.T
    return out
